# revision 9
# baseline (speedup 1.0000x reference)
"""Trainium2 Bass kernel for a custom GRU (B=64, S=512, I=256, H=1024).

Strategy: data-parallel over batch across 8 NeuronCores (8 samples/core).
All matmuls in bf16 (validated ~4e-3 rel err end-to-end vs fp32 reference).

Host-side precompute folds the two input GEMMs into one:
    xzrn = x @ (Wi @ [Wz_x|Wr_x|Wn_x]) + (bi @ [..] + [bz|br|bn])
Device phase A computes xzrn for all timesteps (one big GEMM), bounced
through an HBM scratch. Phase B runs the 512-step recurrence:
    pre_zr = xzrn_t[:, :2H] + h @ [Wz_h|Wr_h]   (identity-matmul preload
                                                 + 8 K-tile accumulate)
    z, r = sigmoid(pre_zr)
    rhT  = (transpose r) * hT
    pre_n = xzrn_t[:, 2H:] + (r*h) @ Wn_h
    h    = (1-z)*tanh(pre_n) + z*h
h is kept both batch-major [8, 1024] (for elementwise) and transposed
[128, 8k*8b] (as matmul stationary operand), with PE transposes bridging.
"""

import numpy as np
import ml_dtypes

import concourse.bacc as bacc
import concourse.tile as tile
import concourse.mybir as mybir
from concourse.bass_utils import run_bass_kernel_spmd

B, S, I_DIM, H = 64, 512, 256, 1024
N_CORES = 8
B_LOC = B // N_CORES  # 8

BF = mybir.dt.bfloat16
F32 = mybir.dt.float32
SIG = mybir.ActivationFunctionType.Sigmoid
TANH = mybir.ActivationFunctionType.Tanh
COPY = mybir.ActivationFunctionType.Copy

npbf = ml_dtypes.bfloat16


def build_gru_nc(s_len=S, b_loc=B_LOC, h=H, i_dim=I_DIM):
    """Build the per-core Bass program (SPMD: same program, per-core data)."""
    nc = bacc.Bacc("TRN2", target_bir_lowering=False, debug=False)
    sb = s_len * b_loc          # flattened (batch, time) rows, batch-major
    ki = i_dim // 128           # K-tiles of the input GEMM
    kh = h // 128               # K-tiles of the recurrent GEMMs
    nz = 3 * h                  # z|r|n concatenated output width
    n_st = sb // 128            # phase-A output row tiles
    czr = (2 * h) // 512        # 512-wide psum chunks for z|r
    cn = h // 512               # 512-wide psum chunks for n

    # Inputs (per-core, host-prepped, bf16)
    xT = nc.dram_tensor("xT", [128, ki, sb], BF, kind="ExternalInput").ap()
    W_eff = nc.dram_tensor("W_eff", [128, ki, nz], BF, kind="ExternalInput").ap()
    beff = nc.dram_tensor("beff", [128, nz], BF, kind="ExternalInput").ap()
    Wzr = nc.dram_tensor("Wzr", [128, kh, 2 * h], BF, kind="ExternalInput").ap()
    Wn = nc.dram_tensor("Wn", [128, kh, h], BF, kind="ExternalInput").ap()
    ident = nc.dram_tensor("ident", [b_loc, b_loc], BF, kind="ExternalInput").ap()
    y = nc.dram_tensor("y", [s_len, b_loc, h], BF, kind="ExternalOutput").ap()

    with tile.TileContext(nc) as tc:
        with tc.tile_pool(name="consts", bufs=1) as consts:
            i8 = consts.tile([b_loc, b_loc], BF, tag="i8")
            nc.sync.dma_start(i8[:], ident)
            wzr_sb = consts.tile([128, kh * 2 * h], BF, tag="wzr")
            nc.sync.dma_start(wzr_sb[:], Wzr.rearrange("p a b -> p (a b)"))
            wn_sb = consts.tile([128, kh * h], BF, tag="wn")
            nc.sync.dma_start(wn_sb[:], Wn.rearrange("p a b -> p (a b)"))

            with tc.tile_pool(name="xzrn_dram", bufs=1, space="DRAM") as dpool:
                xzrn = dpool.tile([s_len, b_loc, nz], BF)

                # ---------------- Phase A: xzrn = x @ W_eff + beff -------------
                with (
                    tc.tile_pool(name="pa_in", bufs=1) as pa_in,
                    tc.tile_pool(name="pa_ps", bufs=4, space="PSUM") as pa_ps,
                    tc.tile_pool(name="pa_st", bufs=4) as pa_st,
                ):
                    xt_sb = pa_in.tile([128, ki * sb], BF, tag="xt")
                    nc.sync.dma_start(xt_sb[:], xT.rearrange("p a b -> p (a b)"))
                    weff_sb = pa_in.tile([128, ki * nz], BF, tag="weff")
                    nc.sync.dma_start(weff_sb[:], W_eff.rearrange("p a b -> p (a b)"))
                    beff_sb = pa_in.tile([128, nz], BF, tag="beff")
                    nc.sync.dma_start(beff_sb[:], beff)

                    for st in range(n_st):
                        bidx = st // (s_len // 128)
                        t0 = (st % (s_len // 128)) * 128
                        for c in range(nz // 512):
                            ps = pa_ps.tile([128, 512], F32, tag="ps")
                            for k in range(ki):
                                nc.tensor.matmul(
                                    ps[:],
                                    xt_sb[:, k * sb + st * 128 : k * sb + (st + 1) * 128],
                                    weff_sb[:, k * nz + c * 512 : k * nz + (c + 1) * 512],
                                    start=(k == 0),
                                    stop=(k == ki - 1),
                                )
                            stg = pa_st.tile([128, 512], BF, tag="stg")
                            nc.vector.tensor_add(
                                stg[:], ps[:], beff_sb[:, c * 512 : (c + 1) * 512]
                            )
                            nc.sync.dma_start(
                                xzrn[t0 : t0 + 128, bidx, c * 512 : (c + 1) * 512],
                                stg[:],
                            )

                # ---------------- Phase B: recurrence --------------------------
                with (
                    tc.tile_pool(name="rx", bufs=4) as rx,
                    tc.tile_pool(name="pz", bufs=1, space="PSUM") as pz,
                    tc.tile_pool(name="pr", bufs=1, space="PSUM") as pr,
                    tc.tile_pool(name="pn", bufs=1, space="PSUM") as pn,
                    tc.tile_pool(name="ptr", bufs=1, space="PSUM") as ptr,
                    tc.tile_pool(name="pth", bufs=1, space="PSUM") as pth,
                    tc.tile_pool(name="el", bufs=2) as el,
                    tc.tile_pool(name="hp", bufs=2) as hp,
                ):
                    h_bm = hp.tile([b_loc, h], BF, tag="h_bm")
                    nc.vector.memset(h_bm[:], 0.0)
                    hT = hp.tile([128, kh * b_loc], BF, tag="hT")
                    nc.vector.memset(hT[:], 0.0)

                    for t in range(s_len):
                        stx = rx.tile([b_loc, nz], BF, tag="stx")
                        nc.sync.dma_start(stx[:], xzrn[t])

                        # pre_z|pre_r = xzrn_t[:, :2H] (identity preload) + h @ Wzr
                        # r chunks first in their own PSUM tile: unblocks the
                        # sigmoid(r) -> transpose -> n-matmul chain while the
                        # z-half still streams on the PE.
                        ps_z = pz.tile([b_loc, h], F32, tag="ps_z")
                        ps_r = pr.tile([b_loc, h], F32, tag="ps_r")
                        r_bf = el.tile([b_loc, h], BF, tag="r_bf")
                        z_bf = el.tile([b_loc, h], BF, tag="z_bf")
                        half = czr // 2
                        for c in list(range(half, czr)) + list(range(half)):
                            ps = ps_r if c >= half else ps_z
                            o0 = (c - half if c >= half else c) * 512
                            nc.tensor.matmul(
                                ps[:, o0 : o0 + 512],
                                i8[:],
                                stx[:, c * 512 : (c + 1) * 512],
                                start=True,
                                stop=False,
                            )
                            for k in range(kh):
                                nc.tensor.matmul(
                                    ps[:, o0 : o0 + 512],
                                    hT[:, k * b_loc : (k + 1) * b_loc],
                                    wzr_sb[:, k * 2 * h + c * 512 : k * 2 * h + (c + 1) * 512],
                                    start=False,
                                    stop=(k == kh - 1),
                                )
                            if c == czr - 1:
                                nc.scalar.activation(r_bf[:], ps_r[:], SIG)
                        nc.scalar.activation(z_bf[:], ps_z[:], SIG)
                        # pre-tanh blend terms: u = 1-z, a = z*h
                        u_bf = el.tile([b_loc, h], BF, tag="u_bf")
                        nc.vector.tensor_scalar(
                            u_bf[:], z_bf[:], -1.0, 1.0,
                            mybir.AluOpType.mult, mybir.AluOpType.add,
                        )
                        a_bf = el.tile([b_loc, h], BF, tag="a_bf")
                        nc.vector.tensor_mul(a_bf[:], z_bf[:], h_bm[:])

                        # rhT = (r transposed) * hT
                        rt_ps = ptr.tile([128, kh * b_loc], BF, tag="rt_ps")
                        for k in range(kh):
                            nc.tensor.transpose(
                                rt_ps[:, k * b_loc : (k + 1) * b_loc],
                                r_bf[:, k * 128 : (k + 1) * 128],
                                i8[:],
                            )
                        rhT = el.tile([128, kh * b_loc], BF, tag="rhT")
                        nc.vector.tensor_mul(rhT[:], rt_ps[:], hT[:])

                        # pre_n = xzrn_t[:, 2H:] (identity preload) + (r*h) @ Wn
                        ps_n = pn.tile([b_loc, h], F32, tag="ps_n")
                        for c in range(cn):
                            nc.tensor.matmul(
                                ps_n[:, c * 512 : (c + 1) * 512],
                                i8[:],
                                stx[:, 2 * h + c * 512 : 2 * h + (c + 1) * 512],
                                start=True,
                                stop=False,
                            )
                            for k in range(kh):
                                nc.tensor.matmul(
                                    ps_n[:, c * 512 : (c + 1) * 512],
                                    rhT[:, k * b_loc : (k + 1) * b_loc],
                                    wn_sb[:, k * h + c * 512 : k * h + (c + 1) * 512],
                                    start=False,
                                    stop=(k == kh - 1),
                                )

                        n_bf = el.tile([b_loc, h], BF, tag="n_bf")
                        nc.scalar.activation(n_bf[:], ps_n[:], TANH)

                        # h_new = n + z * (h - n)
                        d_bf = el.tile([b_loc, h], BF, tag="d_bf")
                        nc.vector.tensor_sub(d_bf[:], h_bm[:], n_bf[:])
                        e_bf = el.tile([b_loc, h], BF, tag="e_bf")
                        nc.vector.tensor_mul(e_bf[:], z_bf[:], d_bf[:])
                        h_bm = hp.tile([b_loc, h], BF, tag="h_bm")
                        nc.vector.tensor_add(h_bm[:], n_bf[:], e_bf[:])

                        # y_t straight out as bf16 (host converts to fp32)
                        nc.sync.dma_start(y[t], h_bm[:])

                        # hT update via PE transposes
                        ht_ps = pth.tile([128, kh * b_loc], BF, tag="ht_ps")
                        for k in range(kh):
                            nc.tensor.transpose(
                                ht_ps[:, k * b_loc : (k + 1) * b_loc],
                                h_bm[:, k * 128 : (k + 1) * 128],
                                i8[:],
                            )
                        hT = hp.tile([128, kh * b_loc], BF, tag="hT")
                        nc.vector.tensor_copy(hT[:], ht_ps[:])

    nc.compile()
    return nc


def prep_core_inputs(x_core, Wi, bi, Wz, bz, Wr, br, Wn, bn, s_len=S, h=H, i_dim=I_DIM):
    """Host-side prep of one core's input dict (all bf16)."""
    b_loc = x_core.shape[0]
    sb = s_len * b_loc
    ki = i_dim // 128
    kh = h // 128
    nz = 3 * h

    Wx = np.concatenate([Wz[:h], Wr[:h], Wn[:h]], axis=1)  # [H, 3H]
    W_eff = (Wi.astype(np.float64) @ Wx.astype(np.float64))  # [I, 3H]
    b_eff = (bi.astype(np.float64) @ Wx.astype(np.float64)
             + np.concatenate([bz, br, bn]).astype(np.float64))  # [3H]

    xT = x_core.reshape(sb, i_dim).T.reshape(ki, 128, sb).transpose(1, 0, 2)
    W_eff_t = W_eff.reshape(ki, 128, nz).transpose(1, 0, 2)
    beff_b = np.broadcast_to(b_eff[None, :], (128, nz))
    Wzr_cat = np.concatenate([Wz[h:], Wr[h:]], axis=1)  # [H, 2H]
    Wzr_t = Wzr_cat.reshape(kh, 128, 2 * h).transpose(1, 0, 2)
    Wn_t = Wn[h:].reshape(kh, 128, h).transpose(1, 0, 2)

    return {
        "xT": np.ascontiguousarray(xT).astype(npbf),
        "W_eff": np.ascontiguousarray(W_eff_t).astype(npbf),
        "beff": np.ascontiguousarray(beff_b).astype(npbf),
        "Wzr": np.ascontiguousarray(Wzr_t).astype(npbf),
        "Wn": np.ascontiguousarray(Wn_t).astype(npbf),
        "ident": np.eye(b_loc, dtype=npbf),
    }


_NC_CACHE = {}


def get_nc():
    if "nc" not in _NC_CACHE:
        _NC_CACHE["nc"] = build_gru_nc()
    return _NC_CACHE["nc"]


def kernel(x, Wi, bi, Wz, bz, Wr, br, Wn, bn):
    x = np.asarray(x)
    nc = get_nc()
    in_maps = [
        prep_core_inputs(
            x[c * B_LOC : (c + 1) * B_LOC], np.asarray(Wi), np.asarray(bi),
            np.asarray(Wz), np.asarray(bz), np.asarray(Wr), np.asarray(br),
            np.asarray(Wn), np.asarray(bn),
        )
        for c in range(N_CORES)
    ]
    res = run_bass_kernel_spmd(nc, in_maps, list(range(N_CORES)), trace=False)
    # y per core: bf16 [S, B_LOC, H] -> fp32 [B_LOC, S, H]
    parts = [
        res.results[c]["y"].astype(np.float32).transpose(1, 0, 2)
        for c in range(N_CORES)
    ]
    output = np.concatenate(parts, axis=0)
    h_final = output[:, -1]
    return output, h_final[None]


# revision 18
# speedup vs baseline: 1.1321x; 1.1321x over previous
"""Trainium2 Bass kernel for a custom GRU (B=64, S=512, I=256, H=1024).

Strategy: data-parallel over batch across 8 NeuronCores (8 samples/core).
All matmuls in bf16 (validated ~4e-3 rel err end-to-end vs fp32 reference).

Host-side precompute folds the two input GEMMs into one:
    xzrn = x @ (Wi @ [Wz_x|Wr_x|Wn_x]) + (bi @ [..] + [bz|br|bn])
Device phase A computes xzrn for all timesteps (one big GEMM), bounced
through an HBM scratch. Phase B runs the 512-step recurrence:
    pre_zr = xzrn_t[:, :2H] + h @ [Wz_h|Wr_h]   (identity-matmul preload
                                                 + 8 K-tile accumulate)
    z, r = sigmoid(pre_zr)
    rhT  = (transpose r) * hT
    pre_n = xzrn_t[:, 2H:] + (r*h) @ Wn_h
    h    = (1-z)*tanh(pre_n) + z*h
h is kept both batch-major [8, 1024] (for elementwise) and transposed
[128, 8k*8b] (as matmul stationary operand), with PE transposes bridging.
"""

import numpy as np
import ml_dtypes

import concourse.bacc as bacc
import concourse.tile as tile
import concourse.mybir as mybir
from concourse.bass_utils import run_bass_kernel_spmd

B, S, I_DIM, H = 64, 512, 256, 1024
N_CORES = 8
B_LOC = B // N_CORES  # 8

BF = mybir.dt.bfloat16
F32 = mybir.dt.float32
SIG = mybir.ActivationFunctionType.Sigmoid
TANH = mybir.ActivationFunctionType.Tanh
COPY = mybir.ActivationFunctionType.Copy

npbf = ml_dtypes.bfloat16


def build_gru_nc(s_len=S, b_loc=B_LOC, h=H, i_dim=I_DIM, s_compute=None):
    """Build the per-core Bass program (SPMD: same program, per-core data).

    s_compute (dev-only): emit only that many recurrence steps while keeping
    identical I/O shapes — lets wall-clock deltas isolate device time.
    """
    if s_compute is None:
        s_compute = s_len
    nc = bacc.Bacc("TRN2", target_bir_lowering=False, debug=False)
    sb = s_len * b_loc          # flattened (batch, time) rows, batch-major
    ki = i_dim // 128           # K-tiles of the input GEMM
    kh = h // 128               # K-tiles of the recurrent GEMMs
    nz = 3 * h                  # z|r|n concatenated output width
    n_st = sb // 128            # phase-A output row tiles
    czr = (2 * h) // 512        # 512-wide psum chunks for z|r
    cn = h // 512               # 512-wide psum chunks for n

    # Inputs (per-core, host-prepped, bf16)
    xT = nc.dram_tensor("xT", [128, ki, sb], BF, kind="ExternalInput").ap()
    W_eff = nc.dram_tensor("W_eff", [128, ki, nz], BF, kind="ExternalInput").ap()
    beff = nc.dram_tensor("beff", [128, nz], BF, kind="ExternalInput").ap()
    Wzr = nc.dram_tensor("Wzr", [128, kh, 2 * h], BF, kind="ExternalInput").ap()
    Wn = nc.dram_tensor("Wn", [128, kh, h], BF, kind="ExternalInput").ap()
    ident = nc.dram_tensor("ident", [b_loc, b_loc], BF, kind="ExternalInput").ap()
    y = nc.dram_tensor("y", [s_len, b_loc, h], F32, kind="ExternalOutput").ap()

    with tile.TileContext(nc) as tc:
        with tc.tile_pool(name="consts", bufs=1) as consts:
            i8 = consts.tile([b_loc, b_loc], BF, tag="i8")
            nc.sync.dma_start(i8[:], ident)
            wzr_sb = consts.tile([128, kh * 2 * h], BF, tag="wzr")
            nc.sync.dma_start(wzr_sb[:], Wzr.rearrange("p a b -> p (a b)"))
            wn_sb = consts.tile([128, kh * h], BF, tag="wn")
            nc.sync.dma_start(wn_sb[:], Wn.rearrange("p a b -> p (a b)"))
            i8f = consts.tile([b_loc, b_loc], F32, tag="i8f")
            nc.scalar.activation(i8f[:], i8[:], COPY)

            with tc.tile_pool(name="xzrn_dram", bufs=1, space="DRAM") as dpool:
                xzrn = dpool.tile([s_len, b_loc, nz], BF)

                # ---------------- Phase A: xzrn = x @ W_eff + beff -------------
                with (
                    tc.tile_pool(name="pa_in", bufs=1) as pa_in,
                    tc.tile_pool(name="pa_ps", bufs=4, space="PSUM") as pa_ps,
                    tc.tile_pool(name="pa_st", bufs=4) as pa_st,
                ):
                    xt_sb = pa_in.tile([128, ki * sb], BF, tag="xt")
                    nc.sync.dma_start(xt_sb[:], xT.rearrange("p a b -> p (a b)"))
                    weff_sb = pa_in.tile([128, ki * nz], BF, tag="weff")
                    nc.sync.dma_start(weff_sb[:], W_eff.rearrange("p a b -> p (a b)"))
                    beff_sb = pa_in.tile([128, nz], BF, tag="beff")
                    nc.sync.dma_start(beff_sb[:], beff)

                    for st in range(n_st):
                        bidx = st // (s_len // 128)
                        t0 = (st % (s_len // 128)) * 128
                        for c in range(nz // 512):
                            ps = pa_ps.tile([128, 512], F32, tag="ps")
                            for k in range(ki):
                                nc.tensor.matmul(
                                    ps[:],
                                    xt_sb[:, k * sb + st * 128 : k * sb + (st + 1) * 128],
                                    weff_sb[:, k * nz + c * 512 : k * nz + (c + 1) * 512],
                                    start=(k == 0),
                                    stop=(k == ki - 1),
                                )
                            stg = pa_st.tile([128, 512], BF, tag="stg")
                            nc.vector.tensor_add(
                                stg[:], ps[:], beff_sb[:, c * 512 : (c + 1) * 512]
                            )
                            nc.sync.dma_start(
                                xzrn[t0 : t0 + 128, bidx, c * 512 : (c + 1) * 512],
                                stg[:],
                            )

                # ---------------- Phase B: recurrence --------------------------
                with (
                    tc.tile_pool(name="rx", bufs=4) as rx,
                    tc.tile_pool(name="pz", bufs=1, space="PSUM") as pz,
                    tc.tile_pool(name="pr", bufs=1, space="PSUM") as pr,
                    tc.tile_pool(name="pn", bufs=1, space="PSUM") as pn,
                    tc.tile_pool(name="ptr", bufs=1, space="PSUM") as ptr,
                    tc.tile_pool(name="pth", bufs=1, space="PSUM") as pth,
                    tc.tile_pool(name="el", bufs=2) as el,
                    tc.tile_pool(name="hp", bufs=2) as hp,
                ):
                    h_bm = hp.tile([b_loc, h], F32, tag="h_bm")
                    nc.vector.memset(h_bm[:], 0.0)
                    hT = hp.tile([128, kh * b_loc], BF, tag="hT")
                    nc.vector.memset(hT[:], 0.0)

                    half = czr // 2

                    stx_tiles = {}

                    def fetch_stx(t):
                        if t < s_len and t not in stx_tiles:
                            stile = rx.tile([b_loc, nz], BF, tag="stx")
                            nc.sync.dma_start(stile[:], xzrn[t])
                            stx_tiles[t] = stile
                        return stx_tiles.get(t)

                    def preload_zr(t):
                        """Allocate + identity-preload next step's z/r psum.

                        Emitted in the previous step's tanh/blend tail so the
                        PE has work there (PE executes its stream in order).
                        """
                        stx = stx_tiles[t]
                        ps_z = pz.tile([b_loc, h], F32, tag="ps_z")
                        ps_r = pr.tile([b_loc, h], F32, tag="ps_r")
                        for c in list(range(half, czr)) + list(range(half)):
                            ps = ps_r if c >= half else ps_z
                            o0 = (c - half if c >= half else c) * 512
                            nc.tensor.matmul(
                                ps[:, o0 : o0 + 512],
                                i8[:],
                                stx[:, c * 512 : (c + 1) * 512],
                                start=True,
                                stop=False,
                            )
                        return stx, ps_z, ps_r

                    fetch_stx(0)
                    fetch_stx(1)
                    nxt = preload_zr(0)

                    for t in range(s_compute):
                        stx, ps_z, ps_r = nxt
                        fetch_stx(t + 2)
                        stx_tiles.pop(t, None)

                        # pre_z|pre_r += h @ [Wz_h|Wr_h]; r chunks first so
                        # sigmoid(r) -> transpose -> n-matmul chain starts
                        # while the z-half still streams on the PE.
                        r_bf = el.tile([b_loc, h], F32, tag="r_bf")
                        z_bf = el.tile([b_loc, h], F32, tag="z_bf")
                        for c in list(range(half, czr)) + list(range(half)):
                            ps = ps_r if c >= half else ps_z
                            o0 = (c - half if c >= half else c) * 512
                            for k in range(kh):
                                nc.tensor.matmul(
                                    ps[:, o0 : o0 + 512],
                                    hT[:, k * b_loc : (k + 1) * b_loc],
                                    wzr_sb[:, k * 2 * h + c * 512 : k * 2 * h + (c + 1) * 512],
                                    start=False,
                                    stop=(k == kh - 1),
                                )
                            if c == czr - 1:
                                nc.scalar.activation(r_bf[:], ps_r[:], SIG)
                        nc.scalar.activation(z_bf[:], ps_z[:], SIG)
                        # pre-tanh blend terms: u = 1-z, a = z*h
                        u_bf = el.tile([b_loc, h], F32, tag="u_bf")
                        nc.vector.tensor_scalar(
                            u_bf[:], z_bf[:], -1.0, 1.0,
                            mybir.AluOpType.mult, mybir.AluOpType.add,
                        )
                        a_bf = el.tile([b_loc, h], F32, tag="a_bf")
                        nc.vector.tensor_mul(a_bf[:], z_bf[:], h_bm[:])

                        # rhT = (r transposed) * hT
                        rt_ps = ptr.tile([128, kh * b_loc], F32, tag="rt_ps")
                        for k in range(kh):
                            nc.tensor.transpose(
                                rt_ps[:, k * b_loc : (k + 1) * b_loc],
                                r_bf[:, k * 128 : (k + 1) * 128],
                                i8f[:],
                            )
                        rhT = el.tile([128, kh * b_loc], BF, tag="rhT")
                        nc.vector.tensor_mul(rhT[:], rt_ps[:], hT[:])

                        # pre_n = xzrn_t[:, 2H:] (identity preload) + (r*h) @ Wn
                        ps_n = pn.tile([b_loc, h], F32, tag="ps_n")
                        for c in range(cn):
                            nc.tensor.matmul(
                                ps_n[:, c * 512 : (c + 1) * 512],
                                i8[:],
                                stx[:, 2 * h + c * 512 : 2 * h + (c + 1) * 512],
                                start=True,
                                stop=False,
                            )
                            for k in range(kh):
                                nc.tensor.matmul(
                                    ps_n[:, c * 512 : (c + 1) * 512],
                                    rhT[:, k * b_loc : (k + 1) * b_loc],
                                    wn_sb[:, k * h + c * 512 : k * h + (c + 1) * 512],
                                    start=False,
                                    stop=(k == kh - 1),
                                )

                        # preload next step's z/r psum here: these PE ops fill
                        # the tanh/blend tail in the PE instruction stream.
                        if t + 1 < s_compute:
                            nxt = preload_zr(t + 1)

                        # tanh + blend pipelined in quarters, h-transposes
                        # interleaved as their input columns complete:
                        # h_new = n*(1-z) + z*h = n*u + a
                        n_bf = el.tile([b_loc, h], F32, tag="n_bf")
                        v_bf = el.tile([b_loc, h], F32, tag="v_bf")
                        h_bm = hp.tile([b_loc, h], F32, tag="h_bm")
                        ht_ps = pth.tile([128, kh * b_loc], F32, tag="ht_ps")
                        nq = 2
                        hq = h // nq
                        kq = kh // nq
                        for p in range(nq):
                            s0 = p * hq
                            nc.scalar.activation(
                                n_bf[:, s0 : s0 + hq], ps_n[:, s0 : s0 + hq], TANH
                            )
                            nc.vector.tensor_mul(
                                v_bf[:, s0 : s0 + hq],
                                n_bf[:, s0 : s0 + hq],
                                u_bf[:, s0 : s0 + hq],
                            )
                            nc.vector.tensor_add(
                                h_bm[:, s0 : s0 + hq],
                                v_bf[:, s0 : s0 + hq],
                                a_bf[:, s0 : s0 + hq],
                            )
                            for k in range(p * kq, (p + 1) * kq):
                                nc.tensor.transpose(
                                    ht_ps[:, k * b_loc : (k + 1) * b_loc],
                                    h_bm[:, k * 128 : (k + 1) * 128],
                                    i8f[:],
                                )

                        # y_t straight out as bf16 (host converts to fp32)
                        nc.sync.dma_start(y[t], h_bm[:])

                        hT = hp.tile([128, kh * b_loc], BF, tag="hT")
                        hhalf = kh * b_loc // 2
                        nc.vector.tensor_copy(hT[:, :hhalf], ht_ps[:, :hhalf])
                        nc.vector.tensor_copy(hT[:, hhalf:], ht_ps[:, hhalf:])

    nc.compile()
    return nc


def prep_core_inputs(x_core, Wi, bi, Wz, bz, Wr, br, Wn, bn, s_len=S, h=H, i_dim=I_DIM):
    """Host-side prep of one core's input dict (all bf16)."""
    b_loc = x_core.shape[0]
    sb = s_len * b_loc
    ki = i_dim // 128
    kh = h // 128
    nz = 3 * h

    Wx = np.concatenate([Wz[:h], Wr[:h], Wn[:h]], axis=1)  # [H, 3H]
    W_eff = (Wi.astype(np.float64) @ Wx.astype(np.float64))  # [I, 3H]
    b_eff = (bi.astype(np.float64) @ Wx.astype(np.float64)
             + np.concatenate([bz, br, bn]).astype(np.float64))  # [3H]

    xT = x_core.reshape(sb, i_dim).T.reshape(ki, 128, sb).transpose(1, 0, 2)
    W_eff_t = W_eff.reshape(ki, 128, nz).transpose(1, 0, 2)
    beff_b = np.broadcast_to(b_eff[None, :], (128, nz))
    Wzr_cat = np.concatenate([Wz[h:], Wr[h:]], axis=1)  # [H, 2H]
    Wzr_t = Wzr_cat.reshape(kh, 128, 2 * h).transpose(1, 0, 2)
    Wn_t = Wn[h:].reshape(kh, 128, h).transpose(1, 0, 2)

    return {
        "xT": np.ascontiguousarray(xT).astype(npbf),
        "W_eff": np.ascontiguousarray(W_eff_t).astype(npbf),
        "beff": np.ascontiguousarray(beff_b).astype(npbf),
        "Wzr": np.ascontiguousarray(Wzr_t).astype(npbf),
        "Wn": np.ascontiguousarray(Wn_t).astype(npbf),
        "ident": np.eye(b_loc, dtype=npbf),
    }


_NC_CACHE = {}


def get_nc():
    if "nc" not in _NC_CACHE:
        _NC_CACHE["nc"] = build_gru_nc()
    return _NC_CACHE["nc"]


def kernel(x, Wi, bi, Wz, bz, Wr, br, Wn, bn):
    x = np.asarray(x)
    nc = get_nc()
    in_maps = [
        prep_core_inputs(
            x[c * B_LOC : (c + 1) * B_LOC], np.asarray(Wi), np.asarray(bi),
            np.asarray(Wz), np.asarray(bz), np.asarray(Wr), np.asarray(br),
            np.asarray(Wn), np.asarray(bn),
        )
        for c in range(N_CORES)
    ]
    res = run_bass_kernel_spmd(nc, in_maps, list(range(N_CORES)), trace=False)
    # y per core: bf16 [S, B_LOC, H] -> fp32 [B_LOC, S, H]
    parts = [
        res.results[c]["y"].astype(np.float32).transpose(1, 0, 2)
        for c in range(N_CORES)
    ]
    output = np.concatenate(parts, axis=0)
    h_final = output[:, -1]
    return output, h_final[None]


# revision 19
# speedup vs baseline: 1087.8995x; 960.9671x over previous
"""Trainium2 Bass kernel for a custom GRU (B=64, S=512, I=256, H=1024).

Strategy: data-parallel over batch across 8 NeuronCores (8 samples/core).
Matmul operands in bf16; gates / hidden state / blend arithmetic kept in
fp32 (measured ~3.3e-3 rel err end-to-end vs the fp32 reference on HW).

Host-side precompute folds the two input GEMMs into one:
    xzrn = x @ (Wi @ [Wz_x|Wr_x|Wn_x]) + (bi @ [..] + [bz|br|bn])
Device phase A computes xzrn for all timesteps (one big GEMM), bounced
through an HBM scratch. Phase B runs the 512-step recurrence:
    pre_zr = xzrn_t[:, :2H] + h @ [Wz_h|Wr_h]   (identity-matmul preload
                                                 + 8 K-tile accumulate)
    z, r = sigmoid(pre_zr)
    rhT  = (transpose r) * hT
    pre_n = xzrn_t[:, 2H:] + (r*h) @ Wn_h
    h    = (1-z)*tanh(pre_n) + z*h
h is kept both batch-major [8, 1024] (for elementwise) and transposed
[128, 8k*8b] (as matmul stationary operand), with PE transposes bridging.
"""

import numpy as np
import ml_dtypes

import concourse.bacc as bacc
import concourse.tile as tile
import concourse.mybir as mybir
from concourse.bass_utils import run_bass_kernel_spmd

B, S, I_DIM, H = 64, 512, 256, 1024
N_CORES = 8
B_LOC = B // N_CORES  # 8

BF = mybir.dt.bfloat16
F32 = mybir.dt.float32
SIG = mybir.ActivationFunctionType.Sigmoid
TANH = mybir.ActivationFunctionType.Tanh
COPY = mybir.ActivationFunctionType.Copy

npbf = ml_dtypes.bfloat16


def build_gru_nc(s_len=S, b_loc=B_LOC, h=H, i_dim=I_DIM, s_compute=None):
    """Build the per-core Bass program (SPMD: same program, per-core data).

    s_compute (dev-only): emit only that many recurrence steps while keeping
    identical I/O shapes — lets wall-clock deltas isolate device time.
    """
    if s_compute is None:
        s_compute = s_len
    nc = bacc.Bacc("TRN2", target_bir_lowering=False, debug=False)
    sb = s_len * b_loc          # flattened (batch, time) rows, batch-major
    ki = i_dim // 128           # K-tiles of the input GEMM
    kh = h // 128               # K-tiles of the recurrent GEMMs
    nz = 3 * h                  # z|r|n concatenated output width
    n_st = sb // 128            # phase-A output row tiles
    czr = (2 * h) // 512        # 512-wide psum chunks for z|r
    cn = h // 512               # 512-wide psum chunks for n

    # Inputs (per-core, host-prepped, bf16)
    xT = nc.dram_tensor("xT", [128, ki, sb], BF, kind="ExternalInput").ap()
    W_eff = nc.dram_tensor("W_eff", [128, ki, nz], BF, kind="ExternalInput").ap()
    beff = nc.dram_tensor("beff", [128, nz], BF, kind="ExternalInput").ap()
    Wzr = nc.dram_tensor("Wzr", [128, kh, 2 * h], BF, kind="ExternalInput").ap()
    Wn = nc.dram_tensor("Wn", [128, kh, h], BF, kind="ExternalInput").ap()
    ident = nc.dram_tensor("ident", [b_loc, b_loc], BF, kind="ExternalInput").ap()
    y = nc.dram_tensor("y", [s_len, b_loc, h], F32, kind="ExternalOutput").ap()

    with tile.TileContext(nc) as tc:
        with tc.tile_pool(name="consts", bufs=1) as consts:
            i8 = consts.tile([b_loc, b_loc], BF, tag="i8")
            nc.sync.dma_start(i8[:], ident)
            wzr_sb = consts.tile([128, kh * 2 * h], BF, tag="wzr")
            nc.sync.dma_start(wzr_sb[:], Wzr.rearrange("p a b -> p (a b)"))
            wn_sb = consts.tile([128, kh * h], BF, tag="wn")
            nc.sync.dma_start(wn_sb[:], Wn.rearrange("p a b -> p (a b)"))
            i8f = consts.tile([b_loc, b_loc], F32, tag="i8f")
            nc.scalar.activation(i8f[:], i8[:], COPY)

            with tc.tile_pool(name="xzrn_dram", bufs=1, space="DRAM") as dpool:
                xzrn = dpool.tile([s_len, b_loc, nz], BF)

                # ---------------- Phase A: xzrn = x @ W_eff + beff -------------
                with (
                    tc.tile_pool(name="pa_in", bufs=1) as pa_in,
                    tc.tile_pool(name="pa_ps", bufs=4, space="PSUM") as pa_ps,
                    tc.tile_pool(name="pa_st", bufs=4) as pa_st,
                ):
                    xt_sb = pa_in.tile([128, ki * sb], BF, tag="xt")
                    nc.sync.dma_start(xt_sb[:], xT.rearrange("p a b -> p (a b)"))
                    weff_sb = pa_in.tile([128, ki * nz], BF, tag="weff")
                    nc.sync.dma_start(weff_sb[:], W_eff.rearrange("p a b -> p (a b)"))
                    beff_sb = pa_in.tile([128, nz], BF, tag="beff")
                    nc.sync.dma_start(beff_sb[:], beff)

                    for st in range(n_st):
                        bidx = st // (s_len // 128)
                        t0 = (st % (s_len // 128)) * 128
                        for c in range(nz // 512):
                            ps = pa_ps.tile([128, 512], F32, tag="ps")
                            for k in range(ki):
                                nc.tensor.matmul(
                                    ps[:],
                                    xt_sb[:, k * sb + st * 128 : k * sb + (st + 1) * 128],
                                    weff_sb[:, k * nz + c * 512 : k * nz + (c + 1) * 512],
                                    start=(k == 0),
                                    stop=(k == ki - 1),
                                )
                            stg = pa_st.tile([128, 512], BF, tag="stg")
                            nc.vector.tensor_add(
                                stg[:], ps[:], beff_sb[:, c * 512 : (c + 1) * 512]
                            )
                            nc.sync.dma_start(
                                xzrn[t0 : t0 + 128, bidx, c * 512 : (c + 1) * 512],
                                stg[:],
                            )

                # ---------------- Phase B: recurrence --------------------------
                with (
                    tc.tile_pool(name="rx", bufs=4) as rx,
                    tc.tile_pool(name="pz", bufs=1, space="PSUM") as pz,
                    tc.tile_pool(name="pr", bufs=1, space="PSUM") as pr,
                    tc.tile_pool(name="pn", bufs=1, space="PSUM") as pn,
                    tc.tile_pool(name="ptr", bufs=1, space="PSUM") as ptr,
                    tc.tile_pool(name="pth", bufs=1, space="PSUM") as pth,
                    tc.tile_pool(name="el", bufs=2) as el,
                    tc.tile_pool(name="hp", bufs=2) as hp,
                ):
                    h_bm = hp.tile([b_loc, h], F32, tag="h_bm")
                    nc.vector.memset(h_bm[:], 0.0)
                    hT = hp.tile([128, kh * b_loc], BF, tag="hT")
                    nc.vector.memset(hT[:], 0.0)

                    half = czr // 2

                    stx_tiles = {}

                    def fetch_stx(t):
                        if t < s_len and t not in stx_tiles:
                            stile = rx.tile([b_loc, nz], BF, tag="stx")
                            nc.sync.dma_start(stile[:], xzrn[t])
                            stx_tiles[t] = stile
                        return stx_tiles.get(t)

                    def preload_zr(t):
                        """Allocate + identity-preload next step's z/r psum.

                        Emitted in the previous step's tanh/blend tail so the
                        PE has work there (PE executes its stream in order).
                        """
                        stx = stx_tiles[t]
                        ps_z = pz.tile([b_loc, h], F32, tag="ps_z")
                        ps_r = pr.tile([b_loc, h], F32, tag="ps_r")
                        for c in list(range(half, czr)) + list(range(half)):
                            ps = ps_r if c >= half else ps_z
                            o0 = (c - half if c >= half else c) * 512
                            nc.tensor.matmul(
                                ps[:, o0 : o0 + 512],
                                i8[:],
                                stx[:, c * 512 : (c + 1) * 512],
                                start=True,
                                stop=False,
                            )
                        return stx, ps_z, ps_r

                    fetch_stx(0)
                    fetch_stx(1)
                    nxt = preload_zr(0)

                    for t in range(s_compute):
                        stx, ps_z, ps_r = nxt
                        fetch_stx(t + 2)
                        stx_tiles.pop(t, None)

                        # pre_z|pre_r += h @ [Wz_h|Wr_h]; r chunks first so
                        # sigmoid(r) -> transpose -> n-matmul chain starts
                        # while the z-half still streams on the PE.
                        r_bf = el.tile([b_loc, h], F32, tag="r_bf")
                        z_bf = el.tile([b_loc, h], F32, tag="z_bf")
                        for c in list(range(half, czr)) + list(range(half)):
                            ps = ps_r if c >= half else ps_z
                            o0 = (c - half if c >= half else c) * 512
                            for k in range(kh):
                                nc.tensor.matmul(
                                    ps[:, o0 : o0 + 512],
                                    hT[:, k * b_loc : (k + 1) * b_loc],
                                    wzr_sb[:, k * 2 * h + c * 512 : k * 2 * h + (c + 1) * 512],
                                    start=False,
                                    stop=(k == kh - 1),
                                )
                            if c == czr - 1:
                                nc.scalar.activation(r_bf[:], ps_r[:], SIG)
                        nc.scalar.activation(z_bf[:], ps_z[:], SIG)
                        # pre-tanh blend terms: u = 1-z, a = z*h
                        u_bf = el.tile([b_loc, h], F32, tag="u_bf")
                        nc.vector.tensor_scalar(
                            u_bf[:], z_bf[:], -1.0, 1.0,
                            mybir.AluOpType.mult, mybir.AluOpType.add,
                        )
                        a_bf = el.tile([b_loc, h], F32, tag="a_bf")
                        nc.vector.tensor_mul(a_bf[:], z_bf[:], h_bm[:])

                        # rhT = (r transposed) * hT
                        rt_ps = ptr.tile([128, kh * b_loc], F32, tag="rt_ps")
                        for k in range(kh):
                            nc.tensor.transpose(
                                rt_ps[:, k * b_loc : (k + 1) * b_loc],
                                r_bf[:, k * 128 : (k + 1) * 128],
                                i8f[:],
                            )
                        rhT = el.tile([128, kh * b_loc], BF, tag="rhT")
                        nc.vector.tensor_mul(rhT[:], rt_ps[:], hT[:])

                        # pre_n = xzrn_t[:, 2H:] (identity preload) + (r*h) @ Wn
                        ps_n = pn.tile([b_loc, h], F32, tag="ps_n")
                        for c in range(cn):
                            nc.tensor.matmul(
                                ps_n[:, c * 512 : (c + 1) * 512],
                                i8[:],
                                stx[:, 2 * h + c * 512 : 2 * h + (c + 1) * 512],
                                start=True,
                                stop=False,
                            )
                            for k in range(kh):
                                nc.tensor.matmul(
                                    ps_n[:, c * 512 : (c + 1) * 512],
                                    rhT[:, k * b_loc : (k + 1) * b_loc],
                                    wn_sb[:, k * h + c * 512 : k * h + (c + 1) * 512],
                                    start=False,
                                    stop=(k == kh - 1),
                                )

                        # preload next step's z/r psum here: these PE ops fill
                        # the tanh/blend tail in the PE instruction stream.
                        if t + 1 < s_compute:
                            nxt = preload_zr(t + 1)

                        # tanh + blend pipelined in quarters, h-transposes
                        # interleaved as their input columns complete:
                        # h_new = n*(1-z) + z*h = n*u + a
                        n_bf = el.tile([b_loc, h], F32, tag="n_bf")
                        v_bf = el.tile([b_loc, h], F32, tag="v_bf")
                        h_bm = hp.tile([b_loc, h], F32, tag="h_bm")
                        ht_ps = pth.tile([128, kh * b_loc], F32, tag="ht_ps")
                        nq = 2
                        hq = h // nq
                        kq = kh // nq
                        for p in range(nq):
                            s0 = p * hq
                            nc.scalar.activation(
                                n_bf[:, s0 : s0 + hq], ps_n[:, s0 : s0 + hq], TANH
                            )
                            nc.vector.tensor_mul(
                                v_bf[:, s0 : s0 + hq],
                                n_bf[:, s0 : s0 + hq],
                                u_bf[:, s0 : s0 + hq],
                            )
                            nc.vector.tensor_add(
                                h_bm[:, s0 : s0 + hq],
                                v_bf[:, s0 : s0 + hq],
                                a_bf[:, s0 : s0 + hq],
                            )
                            for k in range(p * kq, (p + 1) * kq):
                                nc.tensor.transpose(
                                    ht_ps[:, k * b_loc : (k + 1) * b_loc],
                                    h_bm[:, k * 128 : (k + 1) * 128],
                                    i8f[:],
                                )

                        # y_t straight out as bf16 (host converts to fp32)
                        nc.sync.dma_start(y[t], h_bm[:])

                        hT = hp.tile([128, kh * b_loc], BF, tag="hT")
                        hhalf = kh * b_loc // 2
                        nc.vector.tensor_copy(hT[:, :hhalf], ht_ps[:, :hhalf])
                        nc.vector.tensor_copy(hT[:, hhalf:], ht_ps[:, hhalf:])

    nc.compile()
    return nc


def prep_core_inputs(x_core, Wi, bi, Wz, bz, Wr, br, Wn, bn, s_len=S, h=H, i_dim=I_DIM):
    """Host-side prep of one core's input dict (all bf16)."""
    b_loc = x_core.shape[0]
    sb = s_len * b_loc
    ki = i_dim // 128
    kh = h // 128
    nz = 3 * h

    Wx = np.concatenate([Wz[:h], Wr[:h], Wn[:h]], axis=1)  # [H, 3H]
    W_eff = (Wi.astype(np.float64) @ Wx.astype(np.float64))  # [I, 3H]
    b_eff = (bi.astype(np.float64) @ Wx.astype(np.float64)
             + np.concatenate([bz, br, bn]).astype(np.float64))  # [3H]

    xT = x_core.reshape(sb, i_dim).T.reshape(ki, 128, sb).transpose(1, 0, 2)
    W_eff_t = W_eff.reshape(ki, 128, nz).transpose(1, 0, 2)
    beff_b = np.broadcast_to(b_eff[None, :], (128, nz))
    Wzr_cat = np.concatenate([Wz[h:], Wr[h:]], axis=1)  # [H, 2H]
    Wzr_t = Wzr_cat.reshape(kh, 128, 2 * h).transpose(1, 0, 2)
    Wn_t = Wn[h:].reshape(kh, 128, h).transpose(1, 0, 2)

    return {
        "xT": np.ascontiguousarray(xT).astype(npbf),
        "W_eff": np.ascontiguousarray(W_eff_t).astype(npbf),
        "beff": np.ascontiguousarray(beff_b).astype(npbf),
        "Wzr": np.ascontiguousarray(Wzr_t).astype(npbf),
        "Wn": np.ascontiguousarray(Wn_t).astype(npbf),
        "ident": np.eye(b_loc, dtype=npbf),
    }


_NC_CACHE = {}


def get_nc():
    if "nc" not in _NC_CACHE:
        _NC_CACHE["nc"] = build_gru_nc()
    return _NC_CACHE["nc"]


def kernel(x, Wi, bi, Wz, bz, Wr, br, Wn, bn):
    x = np.asarray(x)
    nc = get_nc()
    in_maps = [
        prep_core_inputs(
            x[c * B_LOC : (c + 1) * B_LOC], np.asarray(Wi), np.asarray(bi),
            np.asarray(Wz), np.asarray(bz), np.asarray(Wr), np.asarray(br),
            np.asarray(Wn), np.asarray(bn),
        )
        for c in range(N_CORES)
    ]
    res = run_bass_kernel_spmd(nc, in_maps, list(range(N_CORES)), trace=False)
    # y per core: bf16 [S, B_LOC, H] -> fp32 [B_LOC, S, H]
    parts = [
        res.results[c]["y"].astype(np.float32).transpose(1, 0, 2)
        for c in range(N_CORES)
    ]
    output = np.concatenate(parts, axis=0)
    h_final = output[:, -1]
    return output, h_final[None]
